# revision 9
# baseline (speedup 1.0000x reference)
"""DepTreeLSTM forward on 8 Trainium2 NeuronCores (Bass/Tile).

Forest of T=4096 full binary trees (depth 5, 63 nodes each), TreeLSTM swept
level-by-level (leaves -> root). Trees are data-parallel: 512 trees per core.

Device layout: channels-on-partitions ("transposed") so per-node work is
column-parallel:
  - emb is fed as embT [E=256 rows -> 2 K-chunks of 128, nodes] (bf16)
  - h/c live as [H=128, nodes] bf16 tiles in SBUF, one tile per level
  - within a core, node columns are ordered (block, level, tree, pos), so the
    children of parent column m at level d are exactly columns 2m, 2m+1 of
    level d-1  ->  child gather is a stride-2 access, no indirection.

Per level (M parents, children = level below):
  ht1 = pairsum(h_prev * Ty), ht0 = pairsum(h_prev) - ht1     (type-masked sums)
  ct1 = pairsum(c_prev * Ty), csum = pairsum(c_prev)
  i,o,u   = W_iou.T @ embT + U_iou.T @ [ht0;ht1]          (PE, PSUM accum)
  f0x,f1x = W_f.T @ embT + U_f_w[:, blk].T @ [ht0;ht1]    (X folded in on PE)
  s0 = sig(f0x + bf0), s1 = sig(f1x + bf1)                (ACT, bias fused)
  c = sig(i)*tanh(u) + s0*csum + (s1-s0)*ct1              (select-free c_cell)
  h = sig(o)*tanh(c)
Ty is the per-child type row broadcast down partitions by a replicating DMA.
"""

import math

import numpy as np
import ml_dtypes

import concourse.bass as bass
import concourse.tile as tile
from concourse import mybir
from concourse.bass_utils import run_bass_kernel_spmd

BF16 = ml_dtypes.bfloat16
F32 = np.float32

# ---------------- problem constants (hardcoded) ----------------
T, C, D, E, H = 4096, 2, 5, 256, 128
COUNTS = [C ** (D - d) for d in range(D + 1)]      # [32,16,8,4,2,1]
OFFS = [0, 32, 48, 56, 60, 62]
S = 63
N = T * S
NCORES = 8
TPC = T // NCORES                                   # 512 trees / core
NBLK = 2                                            # tree blocks / core
BT = TPC // NBLK                                    # 256 trees / block
LVL_M = [BT * c for c in COUNTS]                    # [8192,4096,2048,1024,512,256]
BLK_COLS = BT * S                                   # 16128
CORE_COLS = TPC * S                                 # 32256
TY_BLK = 2 * sum(LVL_M[1:])                         # 15872 child slots / block
TY_TOTAL = NBLK * TY_BLK
MC = 512                                            # parent chunk (<= PSUM bank)
LC = 1024                                           # leaf chunk (2 PSUM banks)
SUB = 2048                                          # emb/ty staging sub-slab

SIG = mybir.ActivationFunctionType.Sigmoid
TANH = mybir.ActivationFunctionType.Tanh

LAST_EXEC_NS = None


def split_waits(nc, nop_max=1, keep_max=1):
    """Walrus in this container rejects instructions with too many sem-waits
    (Drain: 0 allowed, NoOp: 1, others: 2). Move excess waits onto inserted
    NoOps, one wait each."""
    n_fix = 0
    for f in nc.m.functions:
        for bb in f.blocks:
            insts = bb.instructions
            i = 0
            while i < len(insts):
                ins = insts[i]
                si = getattr(ins, "sync_info", None)
                ow = list(si.on_wait) if si and si.on_wait else []
                keep = 0 if type(ins).__name__ == "InstDrain" else keep_max
                if len(ow) > keep:
                    extra = ow[:len(ow) - keep]
                    si.on_wait = ow[len(ow) - keep:]
                    k = 0
                    while extra:
                        chunk, extra = extra[:nop_max], extra[nop_max:]
                        nop = mybir.InstNoOp(
                            name=f"I-wsplit-{ins.name}-{k}", engine=ins.engine,
                            ins=[], outs=[])
                        nop.sync_info = type(si)(on_wait=chunk, on_update=[])
                        insts.insert(i, nop)
                        i += 1
                        k += 1
                        n_fix += 1
                i += 1
    return n_fix


def _lvl_off(blk, d):
    return blk * BLK_COLS + sum(LVL_M[:d])


def _ty_off(blk, d):
    return blk * TY_BLK + 2 * sum(LVL_M[1:d])


# weight slot order in wpack [128, 18, 128]
#  0..5 : W_iou (k,o) = (0,0)(1,0)(0,1)(1,1)(0,2)(1,2)
#  6..7 : W_f k0, k1
#  8..13: U_iou (U0_0, U1_0, U0_1, U1_1, U0_2, U1_2)
#  14..17: U_f_w A0, A1, B0, B1   (A = cols 0:128 -> f0, B = cols 128:256 -> f1)


def build_nc(leaf_lc=LC, iou_bufs=1, ep_bufs=4, ty_bufs=4, wk_bufs=4, hc_bufs=2,
             sched=None, sub=1024):
    nc = bass.Bass()
    embt_d = nc.declare_dram_parameter(
        "embt", [128, 2, CORE_COLS], mybir.dt.bfloat16, isOutput=False)
    ty_d = nc.declare_dram_parameter(
        "tyrow", [1, TY_TOTAL], mybir.dt.bfloat16, isOutput=False)
    w_d = nc.declare_dram_parameter(
        "wpack", [128, 18, 128], mybir.dt.bfloat16, isOutput=False)
    b_d = nc.declare_dram_parameter(
        "bpack", [128, 5], mybir.dt.float32, isOutput=False)
    hout_d = nc.declare_dram_parameter(
        "hout", [128, CORE_COLS], mybir.dt.float32, isOutput=True)

    with tile.TileContext(nc) as tc, \
            tc.tile_pool(name="consts", bufs=1) as consts, \
            tc.tile_pool(name="emb", bufs=ep_bufs) as ep, \
            tc.tile_pool(name="ty", bufs=ty_bufs) as typ, \
            tc.tile_pool(name="hc", bufs=hc_bufs) as hc, \
            tc.tile_pool(name="work", bufs=wk_bufs) as wk, \
            tc.tile_pool(name="psum", bufs=iou_bufs, space="PSUM") as psA, \
            tc.tile_pool(name="psumf", bufs=1, space="PSUM") as psB:

        w_t = consts.tile([128, 18, 128], mybir.dt.bfloat16)
        nc.sync.dma_start(out=w_t[:, 0:6], in_=w_d[:, 0:6, :])
        b_t = consts.tile([128, 5], mybir.dt.float32)
        nc.sync.dma_start(out=b_t, in_=b_d[:, :])
        wstage = [0]

        def load_wrest():
            if wstage[0] == 0:
                nc.sync.dma_start(out=w_t[:, 6:18], in_=w_d[:, 6:18, :])
            wstage[0] += 1

        def WS(s):
            return w_t[:, s, :]

        def BI(s):
            return b_t[:, s:s + 1]

        mm = nc.tensor.matmul
        act = nc.scalar.activation

        if sched is None:
            sched = [(blk, d) for blk in range(NBLK) for d in range(D + 1)]
        hprevs = {}
        cprevs = {}
        for blk, d in sched:
            if True:
                h_prev = hprevs.get(blk)
                c_prev = cprevs.get(blk)
                M = LVL_M[d]
                off = _lvl_off(blk, d)
                h_cur = hc.tile([128, M], mybir.dt.bfloat16, tag="h")
                c_cur = hc.tile([128, M], mybir.dt.bfloat16, tag="c")
                nsub = math.ceil(M / sub)
                for s in range(nsub):
                    m0 = s * sub
                    mw = min(sub, M - m0)
                    embt = ep.tile([128, 2, mw], mybir.dt.bfloat16, tag="emb")
                    nc.sync.dma_start(
                        out=embt, in_=embt_d[:, :, off + m0:off + m0 + mw])
                    if d > 0:
                        tyt = typ.tile([128, 2 * mw], mybir.dt.bfloat16, tag="ty")
                        toff = _ty_off(blk, d) + 2 * m0
                        nc.gpsimd.dma_start(
                            out=tyt,
                            in_=ty_d[0:1, toff:toff + 2 * mw].partition_broadcast(128),
                        )
                    step = leaf_lc if d == 0 else MC
                    for c0 in range(m0, m0 + mw, step):
                        mc = min(step, m0 + mw - c0)
                        e0 = embt[:, 0, c0 - m0:c0 - m0 + mc]
                        e1 = embt[:, 1, c0 - m0:c0 - m0 + mc]
                        psi = psA.tile([128, mc], mybir.dt.float32, tag="i")
                        pso = psA.tile([128, mc], mybir.dt.float32, tag="o")
                        psu = psA.tile([128, mc], mybir.dt.float32, tag="u")
                        if d == 0:
                            for hb in range(0, mc, MC):
                                hw_ = min(MC, mc - hb)
                                sl = slice(hb, hb + hw_)
                                mm(psi[:, sl], WS(0), e0[:, sl], start=True, stop=False)
                                mm(psi[:, sl], WS(1), e1[:, sl], start=False, stop=True)
                                mm(pso[:, sl], WS(2), e0[:, sl], start=True, stop=False)
                                mm(pso[:, sl], WS(3), e1[:, sl], start=False, stop=True)
                                mm(psu[:, sl], WS(4), e0[:, sl], start=True, stop=False)
                                mm(psu[:, sl], WS(5), e1[:, sl], start=False, stop=True)
                        else:
                            hp = h_prev[:, 2 * c0:2 * c0 + 2 * mc]
                            cp = c_prev[:, 2 * c0:2 * c0 + 2 * mc]
                            tyv = tyt[:, 2 * (c0 - m0):2 * (c0 - m0) + 2 * mc]
                            hty = wk.tile([128, 2 * mc], mybir.dt.bfloat16, tag="hty")
                            cty = wk.tile([128, 2 * mc], mybir.dt.bfloat16, tag="cty")
                            nc.vector.tensor_mul(hty, hp, tyv)
                            nc.vector.tensor_mul(cty, cp, tyv)
                            ht1 = wk.tile([128, mc], mybir.dt.bfloat16, tag="ht1")
                            hsum = wk.tile([128, mc], mybir.dt.bfloat16, tag="hsum")
                            ct1 = wk.tile([128, mc], mybir.dt.bfloat16, tag="ct1")
                            csum = wk.tile([128, mc], mybir.dt.bfloat16, tag="csum")
                            h3 = hty.rearrange("p (m t) -> p m t", t=2)
                            nc.gpsimd.tensor_add(ht1, h3[:, :, 0], h3[:, :, 1])
                            hp3 = hp.rearrange("p (m t) -> p m t", t=2)
                            nc.gpsimd.tensor_add(hsum, hp3[:, :, 0], hp3[:, :, 1])
                            c3 = cty.rearrange("p (m t) -> p m t", t=2)
                            nc.gpsimd.tensor_add(ct1, c3[:, :, 0], c3[:, :, 1])
                            cp3 = cp.rearrange("p (m t) -> p m t", t=2)
                            nc.vector.tensor_add(csum, cp3[:, :, 0], cp3[:, :, 1])

                            psf0 = psB.tile([128, mc], mybir.dt.float32, tag="f0")
                            psf1 = psB.tile([128, mc], mybir.dt.float32, tag="f1")
                            mm(psi, WS(0), e0, start=True, stop=False)
                            mm(psi, WS(1), e1, start=False, stop=False)
                            mm(psi, WS(8), hsum, start=False, stop=False)
                            mm(psi, WS(9), ht1, start=False, stop=True)
                            mm(pso, WS(2), e0, start=True, stop=False)
                            mm(pso, WS(3), e1, start=False, stop=False)
                            mm(pso, WS(10), hsum, start=False, stop=False)
                            mm(pso, WS(11), ht1, start=False, stop=True)
                            mm(psu, WS(4), e0, start=True, stop=False)
                            mm(psu, WS(5), e1, start=False, stop=False)
                            mm(psu, WS(12), hsum, start=False, stop=False)
                            mm(psu, WS(13), ht1, start=False, stop=True)
                            mm(psf0, WS(6), e0, start=True, stop=False)
                            mm(psf0, WS(7), e1, start=False, stop=False)
                            mm(psf0, WS(14), hsum, start=False, stop=False)
                            mm(psf0, WS(15), ht1, start=False, stop=True)
                            mm(psf1, WS(6), e0, start=True, stop=False)
                            mm(psf1, WS(7), e1, start=False, stop=False)
                            mm(psf1, WS(16), hsum, start=False, stop=False)
                            mm(psf1, WS(17), ht1, start=False, stop=True)

                        si = wk.tile([128, mc], mybir.dt.bfloat16, tag="si")
                        so = wk.tile([128, mc], mybir.dt.bfloat16, tag="so")
                        tu = wk.tile([128, mc], mybir.dt.bfloat16, tag="tu")
                        act(si, psi, SIG, bias=BI(0))
                        act(so, pso, SIG, bias=BI(1))
                        act(tu, psu, TANH, bias=BI(2))
                        cc = c_cur[:, c0:c0 + mc]
                        if d == 0:
                            nc.vector.tensor_mul(cc, si, tu)
                        else:
                            s0 = wk.tile([128, mc], mybir.dt.bfloat16, tag="s0")
                            s1 = wk.tile([128, mc], mybir.dt.bfloat16, tag="s1")
                            sd = wk.tile([128, mc], mybir.dt.bfloat16, tag="sd")
                            act(s0, psf0, SIG, bias=BI(3))
                            act(s1, psf1, SIG, bias=BI(4))
                            nc.vector.tensor_sub(sd, s1, s0)
                            p1 = wk.tile([128, mc], mybir.dt.bfloat16, tag="p1")
                            p2 = wk.tile([128, mc], mybir.dt.bfloat16, tag="p2")
                            p3 = wk.tile([128, mc], mybir.dt.bfloat16, tag="p3")
                            q = wk.tile([128, mc], mybir.dt.bfloat16, tag="q")
                            nc.vector.tensor_mul(p1, si, tu)
                            nc.vector.tensor_mul(p2, s0, csum)
                            nc.vector.tensor_mul(p3, sd, ct1)
                            nc.vector.tensor_add(q, p1, p2)
                            nc.vector.tensor_add(cc, q, p3)
                        tcv = wk.tile([128, mc], mybir.dt.bfloat16, tag="tc")
                        act(tcv, cc, TANH)
                        nc.vector.tensor_mul(h_cur[:, c0:c0 + mc], so, tcv)
                # store this level's h (bf16 -> f32 cast in SWDGE)
                nc.gpsimd.dma_start(out=hout_d[:, off:off + M], in_=h_cur)
                hprevs[blk], cprevs[blk] = h_cur, c_cur
    split_waits(nc)
    return nc


# ---------------- host side ----------------

def _col_perm():
    """perm0[col] -> node index within a core's tree-range (0..TPC*S)."""
    cols = []
    for blk in range(NBLK):
        for d in range(D + 1):
            for t in range(BT):
                tree = blk * BT + t
                base = tree * S + OFFS[d]
                cols.append(np.arange(base, base + COUNTS[d]))
    return np.concatenate(cols)


_NC_CACHE = {}


def _get_nc():
    if "nc" not in _NC_CACHE:
        _NC_CACHE["nc"] = build_nc()
    return _NC_CACHE["nc"]


def prep_in_maps(emb, child_mask, W_iou, U_iou, b_iou, W_f, U_f_w, U_f_b, b_f,
                 children_idx, child_type):
    emb = np.asarray(emb, F32)
    W_iou = np.asarray(W_iou, F32)
    U_iou = np.asarray(U_iou, F32)
    b_iou = np.asarray(b_iou, F32)
    W_f = np.asarray(W_f, F32)
    U_f_w = np.asarray(U_f_w, F32)
    U_f_b = np.asarray(U_f_b, F32)
    b_f = np.asarray(b_f, F32)
    child_type = np.asarray(child_type, np.int32)

    perm0 = _col_perm()

    # wpack: 18 slots of [128,128] lhsT blocks
    slots = [
        W_iou[0:128, 0:128], W_iou[128:256, 0:128],
        W_iou[0:128, 128:256], W_iou[128:256, 128:256],
        W_iou[0:128, 256:384], W_iou[128:256, 256:384],
        W_f[0:128, :], W_f[128:256, :],
        U_iou[0:128, 0:128], U_iou[128:256, 0:128] - U_iou[0:128, 0:128],
        U_iou[0:128, 128:256], U_iou[128:256, 128:256] - U_iou[0:128, 128:256],
        U_iou[0:128, 256:384], U_iou[128:256, 256:384] - U_iou[0:128, 256:384],
        U_f_w[0:128, 0:128], U_f_w[128:256, 0:128] - U_f_w[0:128, 0:128],
        U_f_w[0:128, 128:256] - U_f_w[0:128, 0:128],
        (U_f_w[128:256, 128:256] - U_f_w[0:128, 128:256])
        - (U_f_w[128:256, 0:128] - U_f_w[0:128, 0:128]),
    ]
    wpack = np.stack(slots, axis=1).astype(BF16)          # [128, 18, 128]
    bpack = np.stack([
        b_iou[0:128], b_iou[128:256], b_iou[256:384],
        U_f_b[0:128] + b_f, U_f_b[128:256] + b_f,
    ], axis=1).astype(F32)                                 # [128, 5]

    emb3 = emb.reshape(NCORES, TPC * S, E)
    ct3 = child_type.reshape(NCORES, TPC, S, 2)

    in_maps = []
    for k in range(NCORES):
        emb_core = emb3[k][perm0]                          # [CORE_COLS, E]
        embt = np.ascontiguousarray(
            emb_core.T.reshape(2, 128, CORE_COLS).transpose(1, 0, 2)
        ).astype(BF16)                                     # [128, 2, CORE_COLS]
        typarts = []
        for blk in range(NBLK):
            for d in range(1, D + 1):
                sl = ct3[k, blk * BT:(blk + 1) * BT, OFFS[d]:OFFS[d] + COUNTS[d], :]
                typarts.append(sl.reshape(-1))
        tyrow = np.concatenate(typarts).astype(BF16).reshape(1, TY_TOTAL)
        in_maps.append({
            "embt": embt, "tyrow": tyrow, "wpack": wpack, "bpack": bpack,
        })
    return in_maps


def kernel(**inputs):
    in_maps = prep_in_maps(**inputs)
    nc = _get_nc()
    res = run_bass_kernel_spmd(nc, in_maps, core_ids=list(range(NCORES)))
    global LAST_EXEC_NS
    LAST_EXEC_NS = res.exec_time_ns

    perm0 = _col_perm()
    h = np.empty((N, H), F32)
    h4 = h.reshape(NCORES, TPC * S, H)
    for k in range(NCORES):
        h4[k][perm0] = res.results[k]["hout"].T
    return h

